# revision 16
# baseline (speedup 1.0000x reference)
"""Trainium2 Bass kernel for nn_Block_17033840296551 (GNN message passing block).

Data-parallel over batch: 16 images -> 8 cores x 2 images. Each core runs the
full block on its 2 images with no cross-core communication.

v2: software-pipelined across the two images so every engine queue stays fed:
  emission order = front(0), front(1), gathers+folds(0), tail(0),
  gathers+folds(1), tail(1).  The SWDGE neighbor gathers (drain-rate-bound at
  ~122 GB/s on the single Q0 queue, ~19us per 2304x1KB) of image i overlap the
  other image's matmul phases, keeping the PE out of its low-clock pstates.

Other changes vs v1:
  * f16 residual stream (no f32 x copy); final BN reads f16, writes f32.
  * fsq = Square-activation evacuation of the g1 PSUM (no separate multiply).
  * sim evacuation fused with the per-column 1/||f_m|| scale as a vector
    tensor-tensor multiply straight out of PSUM (featT stays unnormalized).
  * reciprocal_approx_fast for the norm (replaces 6.5us full-precision op).
  * wrap-index DMAs merged (8 per s-pair instead of 32) + log-doubling
    broadcast (3 DMAs instead of 7).
  * tails split by node-half so g2/f1/f2 of half 0 start after fold s1.
"""

import os
import numpy as np

# problem constants (hardcoded per harness contract)
B, C, H, W = 16, 256, 32, 32
N = H * W           # 1024 pixels per image
K = 9
EPS = 1e-5
IMGS_PER_CORE = 2
N_CORES = 8
NEG_BIG = -30000.0

_cache = {}


# --------------------------------------------------------------------------
# host-side preprocessing
# --------------------------------------------------------------------------
def _bn_fold(p):
    g, b, m, v = np.asarray(p, np.float32)
    s = g / np.sqrt(v + EPS)
    t = b - m * s
    return s, t


def _pack_kxm(w_t, part=128):
    """[K, M] -> [part, K//part, M] (partition-major K tiling)."""
    Kd, M = w_t.shape
    kt = Kd // part
    return np.ascontiguousarray(w_t.reshape(kt, part, M).transpose(1, 0, 2))


def _pack_bias(b, part=128):
    n = b.shape[0]
    t = n // part
    return np.ascontiguousarray(b.reshape(t, part).T)  # [part, t]


def _prep_weights(inp):
    f16 = np.float16
    s1, t1 = _bn_fold(inp['g1_bn'])
    Wg1 = s1[:, None] * inp['g1_w']
    s2, t2 = _bn_fold(inp['g2_bn'])
    Wg2 = s2[:, None] * inp['g2_w']
    sf1, tf1 = _bn_fold(inp['f1_bn'])
    Wf1 = sf1[:, None] * inp['f1_w']
    bf1 = sf1 * inp['f1_b'] + tf1
    sf2, tf2 = _bn_fold(inp['f2_bn'])
    Wf2 = sf2[:, None] * inp['f2_w']
    bf2 = sf2 * inp['f2_b'] + tf2
    sb1, tb1 = _bn_fold(inp['b1_bn'])
    Wb1 = sb1[:, None] * inp['b1_w']
    sb2, tb2 = _bn_fold(inp['b2_bn'])
    Wb2 = sb2[:, None, None, None] * inp['b2_w']
    sb3, tb3 = _bn_fold(inp['b3_bn'])
    Wb3 = sb3[:, None] * inp['b3_w']
    sf, tf = _bn_fold(inp['bnf'])

    A = inp['edge_w'][:, :C]
    Bm = inp['edge_w'][:, C:]
    Wp = A - Bm
    Wq = Bm
    bp = inp['edge_b']

    wb2_t = np.zeros((64, 9, 64), f16)
    for dy in range(3):
        for dx in range(3):
            wb2_t[:, dy * 3 + dx, :] = Wb2[:, :, dy, dx].T.astype(f16)

    return {
        'wg1': _pack_kxm(Wg1.T.astype(f16)),           # [128,2,256]
        'wp': _pack_kxm(Wp.T.astype(f16)),             # [128,2,512]
        'wq': _pack_kxm(Wq.T.astype(f16)),             # [128,2,512]
        'wg2': _pack_kxm(Wg2.T.astype(f16)),           # [128,4,256]
        'wf1': _pack_kxm(Wf1.T.astype(f16)),           # [128,2,1024]
        'wf2': _pack_kxm(Wf2.T.astype(f16)),           # [128,8,256]
        'wb1': _pack_kxm(Wb1.T.astype(f16)),           # [128,2,64]
        'wb2': wb2_t,                                   # [64,9,64]
        'wb3': Wb3.T.astype(f16),                       # [64,256]
        'bt1': _pack_bias(t1),                          # [128,2] f32
        'bt2': _pack_bias(t2),
        'bbp': _pack_bias(bp),                          # [128,4]
        'bbf1': _pack_bias(bf1),                        # [128,8]
        'bbf2': _pack_bias(bf2),
        'btb1': np.ascontiguousarray(tb1[:, None].astype(np.float32)),  # [64,1]
        'btb2': np.ascontiguousarray(tb2[:, None].astype(np.float32)),
        'btb3': _pack_bias(tb3),
        'bsf': _pack_bias(sf),
        'btf': _pack_bias(tf),
    }


# --------------------------------------------------------------------------
# device kernel builder
# --------------------------------------------------------------------------
def _build_bass():
    import concourse.bass as bass
    import concourse.mybir as mybir
    from concourse import bacc
    from concourse.tile import TileContext
    from concourse.masks import make_identity

    dt = mybir.dt
    F16 = dt.float16
    F32 = dt.float32
    AF = mybir.ActivationFunctionType
    OP = mybir.AluOpType

    nc = bacc.Bacc()

    # ---- DRAM parameters ----
    x_d = nc.declare_dram_parameter("x", [IMGS_PER_CORE, C, N], F32, isOutput=False)
    wg1_d = nc.declare_dram_parameter("wg1", [128, 2, 256], F16, isOutput=False)
    wp_d = nc.declare_dram_parameter("wp", [128, 2, 512], F16, isOutput=False)
    wq_d = nc.declare_dram_parameter("wq", [128, 2, 512], F16, isOutput=False)
    wg2_d = nc.declare_dram_parameter("wg2", [128, 4, 256], F16, isOutput=False)
    wf1_d = nc.declare_dram_parameter("wf1", [128, 2, 1024], F16, isOutput=False)
    wf2_d = nc.declare_dram_parameter("wf2", [128, 8, 256], F16, isOutput=False)
    wb1_d = nc.declare_dram_parameter("wb1", [128, 2, 64], F16, isOutput=False)
    wb2_d = nc.declare_dram_parameter("wb2", [64, 9, 64], F16, isOutput=False)
    wb3_d = nc.declare_dram_parameter("wb3", [64, 256], F16, isOutput=False)
    bt1_d = nc.declare_dram_parameter("bt1", [128, 2], F32, isOutput=False)
    bt2_d = nc.declare_dram_parameter("bt2", [128, 2], F32, isOutput=False)
    bbp_d = nc.declare_dram_parameter("bbp", [128, 4], F32, isOutput=False)
    bbf1_d = nc.declare_dram_parameter("bbf1", [128, 8], F32, isOutput=False)
    bbf2_d = nc.declare_dram_parameter("bbf2", [128, 2], F32, isOutput=False)
    btb1_d = nc.declare_dram_parameter("btb1", [64, 1], F32, isOutput=False)
    btb2_d = nc.declare_dram_parameter("btb2", [64, 1], F32, isOutput=False)
    btb3_d = nc.declare_dram_parameter("btb3", [128, 2], F32, isOutput=False)
    bsf_d = nc.declare_dram_parameter("bsf", [128, 2], F32, isOutput=False)
    btf_d = nc.declare_dram_parameter("btf", [128, 2], F32, isOutput=False)
    q_drams = [nc.dram_tensor(f"q_dram{i}", [N, 512], F16)
               for i in range(IMGS_PER_CORE)]
    out_d = nc.declare_dram_parameter("out", [IMGS_PER_CORE, C, N], F32, isOutput=True)

    with TileContext(nc) as tc:
        import contextlib
        ctx = contextlib.ExitStack()
        with ctx:
            consts = ctx.enter_context(tc.tile_pool(name="consts", bufs=1))
            pool_xc = ctx.enter_context(tc.tile_pool(name="xc", bufs=2))
            pool_feat = ctx.enter_context(tc.tile_pool(name="feat", bufs=2))
            pool_sm = ctx.enter_context(tc.tile_pool(name="sm", bufs=2))
            pool_simI = ctx.enter_context(tc.tile_pool(name="simI", bufs=3))
            pool_f1 = ctx.enter_context(tc.tile_pool(name="f1o", bufs=2))
            pool_idx = ctx.enter_context(tc.tile_pool(name="idx", bufs=2))
            pool_q = ctx.enter_context(tc.tile_pool(name="q", bufs=2))
            pool_p = ctx.enter_context(tc.tile_pool(name="p", bufs=2))
            pool_gath = ctx.enter_context(tc.tile_pool(name="gath", bufs=4))
            pool_e = ctx.enter_context(tc.tile_pool(name="e", bufs=2))
            pool_h = ctx.enter_context(tc.tile_pool(name="h", bufs=2))
            pool_bn = ctx.enter_context(tc.tile_pool(name="bn", bufs=2))
            pool_o = ctx.enter_context(tc.tile_pool(name="o", bufs=2))
            pool_tmp = ctx.enter_context(tc.tile_pool(name="tmp", bufs=3))
            psum = ctx.enter_context(tc.tile_pool(name="psum", bufs=3, space="PSUM"))
            psumT = ctx.enter_context(tc.tile_pool(name="psumT", bufs=2, space="PSUM"))
            psum1 = ctx.enter_context(tc.tile_pool(name="psum1", bufs=1, space="PSUM"))
            psum64 = ctx.enter_context(tc.tile_pool(name="psum64", bufs=2, space="PSUM"))

            def load(name, shape, dtype, src):
                t = consts.tile(shape, dtype, name=name)
                nc.sync.dma_start(out=t[:], in_=src[:])
                return t

            # wave 1: only what front(0) needs (keeps the sync queue clear
            # for q stores + wrap DMAs that gate the first gathers)
            wg1 = load("wg1s", [128, 2, 256], F16, wg1_d)
            bt1 = load("bt1s", [128, 2], F32, bt1_d)
            wq = load("wqs", [128, 2, 512], F16, wq_d)
            wp = load("wps", [128, 2, 512], F16, wp_d)
            wv = {}

            def load_wave2():
                wv['wg2'] = load("wg2s", [128, 4, 256], F16, wg2_d)
                wv['wf1'] = load("wf1s", [128, 2, 1024], F16, wf1_d)
                wv['wf2'] = load("wf2s", [128, 8, 256], F16, wf2_d)
                wv['wb1'] = load("wb1s", [128, 2, 64], F16, wb1_d)
                wv['wb2'] = load("wb2s", [64, 9, 64], F16, wb2_d)
                wv['wb3'] = load("wb3s", [64, 256], F16, wb3_d)
                wv['bt2'] = load("bt2s", [128, 2], F32, bt2_d)
                wv['bbp'] = load("bbps", [128, 4], F32, bbp_d)
                wv['bbf1'] = load("bbf1s", [128, 8], F32, bbf1_d)
                wv['bbf2'] = load("bbf2s", [128, 2], F32, bbf2_d)
                wv['btb1'] = load("btb1s", [64, 1], F32, btb1_d)
                wv['btb2'] = load("btb2s", [64, 1], F32, btb2_d)
                wv['btb3'] = load("btb3s", [128, 2], F32, btb3_d)
                wv['bsf'] = load("bsfs", [128, 2], F32, bsf_d)
                wv['btf'] = load("btfs", [128, 2], F32, btf_d)

            # SWDGE warm-up: a dummy gather pays the one-time pool-config
            # + ucode setup cost before the real gathers need the engine.
            warm_idx = consts.tile([128, 8], dt.int16, name="warm_idx")
            nc.vector.memset(warm_idx[:], 0)
            warm_out = consts.tile([128, 1, 512], F16, name="warm_out")
            nc.gpsimd.dma_gather(
                out_ap=warm_out[:], in_ap=q_drams[0][:], idxs_ap=warm_idx[:],
                num_idxs=128, num_idxs_reg=128, elem_size=512,
                transpose=False, single_packet=False)

            ident = consts.tile([128, 128], F16, name="ident")
            make_identity(nc, ident[:])
            negid = consts.tile([128, 128], F16, name="negid")
            nc.scalar.activation(out=negid[:], in_=ident[:], func=AF.Copy,
                                 scale=NEG_BIG)
            ones = consts.tile([128, 128], F16, name="ones")
            nc.vector.memset(ones[:], 1.0)
            # idbig[k, f] = 1 iff f == k + 384 (shifted identity for diag-kill)
            idbig = consts.tile([128, 1024], F16, name="idbig")
            nc.vector.memset(idbig[:], 0.0)
            nc.gpsimd.affine_select(
                out=idbig[:], in_=idbig[:],
                compare_op=mybir.AluOpType.not_equal, fill=1.0,
                base=384, pattern=[[-1, 1024]], channel_multiplier=1)

            # per-image state
            st = [dict() for _ in range(IMGS_PER_CORE)]

            # ============ phase functions ============
            def loadx(img):
                d = st[img]
                xc = pool_xc.tile([128, 2, N], F16, name=f"xc{img}", tag="xc")
                if img == 0:
                    for t in range(2):
                        # cast f32->f16 during DMA (SWDGE queue idle at t=0)
                        nc.gpsimd.dma_start(out=xc[:, t, :],
                                            in_=x_d[img, t * 128:(t + 1) * 128, :])
                else:
                    # keep the SWDGE queue free for img0's gathers
                    for t in range(2):
                        x32 = pool_xc.tile([128, N], F32, name=f"x32_{t}",
                                           tag="x32")
                        nc.sync.dma_start(out=x32[:],
                                          in_=x_d[img, t * 128:(t + 1) * 128, :])
                        if t == 0:
                            nc.scalar.activation(out=xc[:, t, :], in_=x32[:],
                                                 func=AF.Copy)
                        else:
                            nc.vector.tensor_copy(out=xc[:, t, :], in_=x32[:])
                wrapped = pool_idx.tile([128, 8, 64], dt.int16,
                                        name=f"wrapped{img}", tag="wrapped")
                d['xc'] = xc
                d['wrapped'] = wrapped

            def front(img):
                d = st[img]
                xc = d['xc']
                # ---- g1 -> featT (unnormalized) + fsq (Square evac) ----
                featT = pool_feat.tile([128, 2, N], F16, name=f"featT{img}", tag="featT")
                fsq = pool_sm.tile([128, 2, N], F16, name=f"fsq{img}", tag="fsq")
                for to in range(2):
                    for nb in range(2):
                        ps = psum.tile([128, 512], F32, name="ps_g1", tag="ps")
                        for kt in range(2):
                            nc.tensor.matmul(
                                ps[:], lhsT=wg1[:, kt, to * 128:(to + 1) * 128],
                                rhs=xc[:, kt, nb * 512:(nb + 1) * 512],
                                start=(kt == 0), stop=(kt == 1))
                        sl = slice(nb * 512, (nb + 1) * 512)
                        nc.scalar.activation(
                            out=fsq[:, to, sl], in_=ps[:],
                            func=AF.Square, bias=bt1[:, to:to + 1])
                        nc.scalar.activation(
                            out=featT[:, to, sl], in_=ps[:],
                            func=AF.Identity, bias=bt1[:, to:to + 1])

                # ---- n2 partial (matmul + fast reciprocal) ----
                invn = pool_sm.tile([1, N], F16, name=f"invn{img}", tag="invn")
                rn2 = pool_sm.tile([1, N], F32, name=f"rn2{img}", tag="rn2")
                for nb in range(2):
                    ps1 = psum1.tile([1, 512], F32, name="ps_n2", tag="ps1")
                    for kt in range(2):
                        nc.tensor.matmul(
                            ps1[:], lhsT=ones[:, 0:1],
                            rhs=fsq[:, kt, nb * 512:(nb + 1) * 512],
                            start=(kt == 0), stop=(kt == 1))
                    nc.vector.reciprocal_approx_fast(
                        out=rn2[:, nb * 512:(nb + 1) * 512], in_=ps1[:])

                # ---- invn/invnb broadcast (feeds sim evac; keep early) ----
                nc.scalar.activation(out=invn[:], in_=rn2[:], func=AF.Sqrt)
                invnb = pool_sm.tile([128, N], F16, name=f"invnb{img}", tag="invnb")
                for nb in range(2):
                    psb = psum.tile([128, 512], F32, name="ps_bc", tag="ps")
                    nc.tensor.matmul(psb[:], lhsT=ones[0:1, :],
                                     rhs=invn[:, nb * 512:(nb + 1) * 512],
                                     start=True, stop=True)
                    nc.scalar.activation(out=invnb[:, nb * 512:(nb + 1) * 512],
                                         in_=psb[:], func=AF.Copy)

                # ---- q (node-partitioned) -> q_sb + q_dram (gates gathers) ----
                q_dram = q_drams[img]
                q_sb = pool_q.tile([128, 8, 512], F16, name=f"q_sb{img}",
                                   tag="q_sb")
                for nt in range(8):
                    ps = psum.tile([128, 512], F32, name="ps_q", tag="ps")
                    for kt in range(2):
                        nc.tensor.matmul(
                            ps[:], lhsT=featT[:, kt, nt * 128:(nt + 1) * 128],
                            rhs=wq[:, kt, :], start=(kt == 0), stop=(kt == 1))
                    nc.scalar.activation(out=q_sb[:, nt, :], in_=ps[:],
                                         func=AF.Copy)
                    nc.sync.dma_start(out=q_dram[nt * 128:(nt + 1) * 128, :],
                                      in_=q_sb[:, nt, :])
                d['q_sb'] = q_sb

                # ---- p (node-partitioned, no bias yet) ----
                p_np = pool_p.tile([128, 8, 512], F16, name=f"p_np{img}",
                                   tag="p_np")
                for nt in range(8):
                    ps = psum.tile([128, 512], F32, name="ps_p", tag="ps")
                    for kt in range(2):
                        nc.tensor.matmul(
                            ps[:], lhsT=featT[:, kt, nt * 128:(nt + 1) * 128],
                            rhs=wp[:, kt, :], start=(kt == 0), stop=(kt == 1))
                    nc.scalar.activation(out=p_np[:, nt, :], in_=ps[:],
                                         func=AF.Copy)
                d['p_np'] = p_np

                # ---- sim scores (diag killed), scaled by invnb on evac ----
                ixbuf = pool_idx.tile([128, 8, 8], dt.uint16,
                                      name=f"ixbuf{img}", tag="ixbuf")
                ixi = ixbuf[:].bitcast(dt.int16)
                wrapped = d['wrapped']
                # wrapped[p16, I, 8k+g] = ixbuf[16g+p16, I, k]
                wview = wrapped[0:16, :, :].rearrange(
                    "p i (k g) -> p i k g", k=8, g=8)
                wflat = wrapped[:].rearrange("p a b -> p (a b)")
                for I in range(8):
                    simI = pool_simI.tile([128, N], F16, name="simI", tag="simI")
                    for cb in range(2):
                        has_diag = (cb == I // 4)
                        ps = psum.tile([128, 512], F32, name="ps_sim", tag="ps")
                        for kt in range(2):
                            nc.tensor.matmul(
                                ps[:], lhsT=featT[:, kt, I * 128:(I + 1) * 128],
                                rhs=featT[:, kt, cb * 512:(cb + 1) * 512],
                                start=(kt == 0),
                                stop=(kt == 1 and not has_diag))
                        if has_diag:
                            off = I * 128 - cb * 512
                            nc.tensor.matmul(ps[:], lhsT=negid[:],
                                             rhs=idbig[:, 384 - off:896 - off],
                                             start=False, stop=True)
                        # evac fused with per-column 1/||f_m|| scale
                        nc.vector.tensor_mul(
                            simI[:, cb * 512:(cb + 1) * 512], ps[:],
                            invnb[:, cb * 512:(cb + 1) * 512])
                    mx = pool_tmp.tile([128, 8], F16, name="mx", tag="mx")
                    nc.vector.max(out=mx[:], in_=simI[:])
                    nc.vector.max_index(out=ixbuf[:, I, :],
                                        in_max=mx[:],
                                        in_values=simI[:])
                    if I % 2 == 1:
                        # wrap this I-pair into the 16-partition layout
                        isl = slice(I - 1, I + 1)
                        for g in range(8):
                            nc.sync.dma_start(
                                out=wview[:, isl, :, g],
                                in_=ixi[16 * g:16 * (g + 1), isl, :])
                        # log-doubling broadcast to 128 partitions
                        csl = slice(64 * (I - 1), 64 * (I + 1))
                        nc.sync.dma_start(out=wflat[16:32, csl],
                                          in_=wflat[0:16, csl])
                        nc.sync.dma_start(out=wflat[32:64, csl],
                                          in_=wflat[0:32, csl])
                        nc.sync.dma_start(out=wflat[64:128, csl],
                                          in_=wflat[0:64, csl])
                        # issue the pair's gathers immediately (gpsimd)
                        gather_issue(img, I - 1)
                        gather_issue(img, I)



            def gather_issue(img, I):
                """Issue the NT SWDGE gather for 128-node block I."""
                d = st[img]
                wrapped = d['wrapped']
                wflat = wrapped[:].rearrange("p a b -> p (a b)")
                go = pool_gath.tile([128, 8, 512], F16, name="go", tag="go")
                nc.gpsimd.dma_gather(
                    out_ap=go[:], in_ap=q_drams[img][:],
                    idxs_ap=wflat[:, 64 * I:64 * (I + 1)],
                    num_idxs=1024, num_idxs_reg=1024, elem_size=512,
                    transpose=False, single_packet=False)
                d[f'go{I}'] = go

            def fold_join(img, I):
                """Max fold over the 8 gathered rows + self + p join."""
                d = st[img]
                p_np = d['p_np']
                q_sb = d['q_sb']
                go = d.pop(f'go{I}')
                gf = go[:].rearrange("p a b -> p (a b)")
                nc.vector.tensor_max(gf[:, 2048:4096], gf[:, 0:2048],
                                     gf[:, 2048:4096])
                nc.vector.tensor_max(gf[:, 3072:4096], gf[:, 2048:3072],
                                     gf[:, 3072:4096])
                nc.vector.tensor_max(gf[:, 3584:4096], gf[:, 3072:3584],
                                     gf[:, 3584:4096])
                # self neighbor + p join; bias+relu happen in transpose evac
                nc.vector.tensor_max(gf[:, 3584:4096], gf[:, 3584:4096],
                                     q_sb[:, I, :])
                nc.vector.tensor_add(p_np[:, I, :], p_np[:, I, :],
                                     gf[:, 3584:4096])

            def transpose_e(img, h):
                """PE-transpose e half h (nodes 512h..512h+512) into eT
                [128c, 4cb, N]; evac applies edge bias + relu per channel."""
                d = st[img]
                p_np = d['p_np']
                if h == 0:
                    d['eT'] = pool_e.tile([128, 4, N], F16, name=f"eT{img}",
                                          tag="eT")
                eT = d['eT']
                for cb in range(4):
                    psT = psumT.tile([128, 512], F16, name="psT", tag="psT")
                    for j in range(4):
                        I = 4 * h + j
                        nc.tensor.transpose(
                            out=psT[:, j * 128:(j + 1) * 128],
                            in_=p_np[:, I, cb * 128:(cb + 1) * 128],
                            identity=ident[:])
                    nc.scalar.activation(
                        out=eT[:, cb, h * 512:(h + 1) * 512], in_=psT[:],
                        func=AF.Relu, bias=wv['bbp'][:, cb:cb + 1])

            def tail_gf(img, nb):
                """g2 + residual, f1, f2 + residual for one node-half."""
                d = st[img]
                xc = d['xc']
                eT = d['eT']
                sl = slice(nb * 512, (nb + 1) * 512)
                if nb == 0:
                    d['h'] = pool_h.tile([128, 2, N], F16, name=f"h{img}", tag="h")
                h = d['h']
                f1o = pool_f1.tile([128, 8, 512], F16, name="f1o", tag="f1o")
                # g2 + residual
                for to in range(2):
                    ps = psum.tile([128, 512], F32, name="ps_g2", tag="ps")
                    for kt in range(4):
                        nc.tensor.matmul(
                            ps[:], lhsT=wv['wg2'][:, kt, to * 128:(to + 1) * 128],
                            rhs=eT[:, kt, sl],
                            start=(kt == 0), stop=(kt == 3))
                    tmp = pool_tmp.tile([128, 512], F16, name="g2tmp", tag="evtmp")
                    nc.scalar.activation(out=tmp[:], in_=ps[:],
                                         func=AF.Identity, bias=wv['bt2'][:, to:to + 1])
                    nc.vector.tensor_add(h[:, to, sl], tmp[:], xc[:, to, sl])
                # f1 (relu evac)
                for to in range(8):
                    ps = psum.tile([128, 512], F32, name="ps_f1", tag="ps")
                    for kt in range(2):
                        nc.tensor.matmul(
                            ps[:], lhsT=wv['wf1'][:, kt, to * 128:(to + 1) * 128],
                            rhs=h[:, kt, sl],
                            start=(kt == 0), stop=(kt == 1))
                    nc.scalar.activation(
                        out=f1o[:, to, :], in_=ps[:],
                        func=AF.Relu, bias=wv['bbf1'][:, to:to + 1])
                # f2 + residual (h -> h2 in place)
                for to in range(2):
                    ps = psum.tile([128, 512], F32, name="ps_f2", tag="ps")
                    for kt in range(8):
                        nc.tensor.matmul(
                            ps[:], lhsT=wv['wf2'][:, kt, to * 128:(to + 1) * 128],
                            rhs=f1o[:, kt, :],
                            start=(kt == 0), stop=(kt == 7))
                    tmp = pool_tmp.tile([128, 512], F16, name="f2tmp", tag="evtmp")
                    nc.scalar.activation(out=tmp[:], in_=ps[:],
                                         func=AF.Identity, bias=wv['bbf2'][:, to:to + 1])
                    nc.vector.tensor_add(h[:, to, sl], tmp[:], h[:, to, sl])

            def tail_bneck(img):
                """bottleneck + final BN + store (whole image)."""
                d = st[img]
                xc = d['xc']
                h = d['h']  # h2 now
                b1o = pool_bn.tile([64, N], F16, name=f"b1o{img}", tag="b1o")
                for nb in range(2):
                    ps = psum64.tile([64, 512], F32, name="ps_b1", tag="ps64")
                    for kt in range(2):
                        nc.tensor.matmul(
                            ps[:], lhsT=wv['wb1'][:, kt, :],
                            rhs=h[:, kt, nb * 512:(nb + 1) * 512],
                            start=(kt == 0), stop=(kt == 1))
                    nc.scalar.activation(out=b1o[:, nb * 512:(nb + 1) * 512],
                                         in_=ps[:], func=AF.Relu, bias=wv['btb1'][:, 0:1])
                pad = pool_bn.tile([64, 34 * 34], F16, name=f"pad{img}", tag="pad")
                nc.vector.memset(pad[:], 0.0)
                pad3 = pad[:].rearrange("p (r c) -> p r c", r=34)
                b1v = b1o[:].rearrange("p (r c) -> p r c", r=32)
                nc.vector.tensor_copy(pad3[:, 1:33, 1:33], b1v)
                b2o = pool_bn.tile([64, N], F16, name=f"b2o{img}", tag="b2o")
                for nb in range(2):
                    ps = psum64.tile([64, 512], F32, name="ps_b2", tag="ps64")
                    for tap in range(9):
                        dy, dx = tap // 3, tap % 3
                        rhs = pad3[:, 16 * nb + dy:16 * nb + dy + 16, dx:dx + 32]
                        nc.tensor.matmul(ps[:], lhsT=wv['wb2'][:, tap, :], rhs=rhs,
                                         start=(tap == 0), stop=(tap == 8))
                    nc.scalar.activation(out=b2o[:, nb * 512:(nb + 1) * 512],
                                         in_=ps[:], func=AF.Relu, bias=wv['btb2'][:, 0:1])
                for to in range(2):
                    out32 = pool_o.tile([128, N], F32, name="out32", tag="o32")
                    d[f'out32_{to}'] = out32
                    for nb in range(2):
                        ps = psum.tile([128, 512], F32, name="ps_b3", tag="ps")
                        nc.tensor.matmul(
                            ps[:], lhsT=wv['wb3'][:, to * 128:(to + 1) * 128],
                            rhs=b2o[:, nb * 512:(nb + 1) * 512],
                            start=True, stop=True)
                        tmp = pool_tmp.tile([128, 512], F16, name="b3tmp", tag="evtmp")
                        nc.scalar.activation(out=tmp[:], in_=ps[:],
                                             func=AF.Identity,
                                             bias=wv['btb3'][:, to:to + 1])
                        sl = slice(nb * 512, (nb + 1) * 512)
                        # o3 = tmp + h2 ; fin = o3 + x  (both f16, into h)
                        nc.vector.tensor_add(h[:, to, sl], tmp[:], h[:, to, sl])
                        nc.vector.tensor_add(h[:, to, sl], h[:, to, sl],
                                             xc[:, to, sl])
                        nc.scalar.activation(out=out32[:, sl],
                                             in_=h[:, to, sl],
                                             func=AF.Identity,
                                             scale=wv['bsf'][:, to:to + 1],
                                             bias=wv['btf'][:, to:to + 1])
                    nc.sync.dma_start(out=out_d[img, to * 128:(to + 1) * 128, :],
                                      in_=out32[:])

            # ============ emission (software pipeline over 2 images) ============
            loadx(0)
            front(0)
            load_wave2()
            loadx(1)
            front(1)
            for I in range(8):
                fold_join(0, I)
            transpose_e(0, 0)
            tail_gf(0, 0)
            transpose_e(0, 1)
            tail_gf(0, 1)
            for I in range(4):
                fold_join(1, I)
            tail_bneck(0)
            for I in range(4, 8):
                fold_join(1, I)
            transpose_e(1, 0)
            tail_gf(1, 0)
            transpose_e(1, 1)
            tail_gf(1, 1)
            tail_bneck(1)

    nc.finalize()
    return nc


# --------------------------------------------------------------------------
# entry point
# --------------------------------------------------------------------------
def kernel(**inputs):
    inp = {k: np.asarray(v) for k, v in inputs.items()}
    w = _prep_weights(inp)

    if 'nc' not in _cache:
        _cache['nc'] = _build_bass()
    nc = _cache['nc']

    x = inp['x'].astype(np.float32).reshape(B, C, N)
    in_maps = []
    for c in range(N_CORES):
        m = {'x': np.ascontiguousarray(x[c * 2:(c + 1) * 2])}
        m.update({k: v for k, v in w.items()})
        in_maps.append(m)

    from concourse.bass_utils import run_bass_kernel_spmd
    trace = bool(os.environ.get("KBENCH_TRACE"))
    res = run_bass_kernel_spmd(nc, in_maps, core_ids=list(range(N_CORES)),
                               trace=trace)
    _cache['exec_time_ns'] = res.exec_time_ns
    _cache['results'] = res
    out = np.zeros((B, C, N), np.float32)
    for c in range(N_CORES):
        out[c * 2:(c + 1) * 2] = res.results[c]['out']
    return out.reshape(B, C, H, W)


# revision 24
# speedup vs baseline: 1.0034x; 1.0034x over previous
"""Trainium2 Bass kernel for nn_Block_17033840296551 (GNN message passing block).

Data-parallel over batch: 16 images -> 8 cores x 2 images. Each core runs the
full block on its 2 images with no cross-core communication.

v2: software-pipelined across the two images so every engine queue stays fed:
  emission order = front(0), front(1), gathers+folds(0), tail(0),
  gathers+folds(1), tail(1).  The SWDGE neighbor gathers (drain-rate-bound at
  ~122 GB/s on the single Q0 queue, ~19us per 2304x1KB) of image i overlap the
  other image's matmul phases, keeping the PE out of its low-clock pstates.

Other changes vs v1:
  * f16 residual stream (no f32 x copy); final BN reads f16, writes f32.
  * fsq = Square-activation evacuation of the g1 PSUM (no separate multiply).
  * sim evacuation fused with the per-column 1/||f_m|| scale as a vector
    tensor-tensor multiply straight out of PSUM (featT stays unnormalized).
  * reciprocal_approx_fast for the norm (replaces 6.5us full-precision op).
  * wrap-index DMAs merged (8 per s-pair instead of 32) + log-doubling
    broadcast (3 DMAs instead of 7).
  * tails split by node-half so g2/f1/f2 of half 0 start after fold s1.
"""

import os
import numpy as np

# problem constants (hardcoded per harness contract)
B, C, H, W = 16, 256, 32, 32
N = H * W           # 1024 pixels per image
K = 9
EPS = 1e-5
IMGS_PER_CORE = 2
N_CORES = 8
NEG_BIG = -30000.0

_cache = {}


# --------------------------------------------------------------------------
# host-side preprocessing
# --------------------------------------------------------------------------
def _bn_fold(p):
    g, b, m, v = np.asarray(p, np.float32)
    s = g / np.sqrt(v + EPS)
    t = b - m * s
    return s, t


def _pack_kxm(w_t, part=128):
    """[K, M] -> [part, K//part, M] (partition-major K tiling)."""
    Kd, M = w_t.shape
    kt = Kd // part
    return np.ascontiguousarray(w_t.reshape(kt, part, M).transpose(1, 0, 2))


def _pack_bias(b, part=128):
    n = b.shape[0]
    t = n // part
    return np.ascontiguousarray(b.reshape(t, part).T)  # [part, t]


def _prep_weights(inp):
    f16 = np.float16
    s1, t1 = _bn_fold(inp['g1_bn'])
    Wg1 = s1[:, None] * inp['g1_w']
    s2, t2 = _bn_fold(inp['g2_bn'])
    Wg2 = s2[:, None] * inp['g2_w']
    sf1, tf1 = _bn_fold(inp['f1_bn'])
    Wf1 = sf1[:, None] * inp['f1_w']
    bf1 = sf1 * inp['f1_b'] + tf1
    sf2, tf2 = _bn_fold(inp['f2_bn'])
    Wf2 = sf2[:, None] * inp['f2_w']
    bf2 = sf2 * inp['f2_b'] + tf2
    sb1, tb1 = _bn_fold(inp['b1_bn'])
    Wb1 = sb1[:, None] * inp['b1_w']
    sb2, tb2 = _bn_fold(inp['b2_bn'])
    Wb2 = sb2[:, None, None, None] * inp['b2_w']
    sb3, tb3 = _bn_fold(inp['b3_bn'])
    Wb3 = sb3[:, None] * inp['b3_w']
    sf, tf = _bn_fold(inp['bnf'])

    A = inp['edge_w'][:, :C]
    Bm = inp['edge_w'][:, C:]
    Wp = A - Bm
    Wq = Bm
    bp = inp['edge_b']

    wb2_t = np.zeros((64, 9, 64), f16)
    for dy in range(3):
        for dx in range(3):
            wb2_t[:, dy * 3 + dx, :] = Wb2[:, :, dy, dx].T.astype(f16)

    return {
        'wg1': _pack_kxm(Wg1.T.astype(f16)),           # [128,2,256]
        'wp': _pack_kxm(Wp.T.astype(f16)),             # [128,2,512]
        'wq': _pack_kxm(Wq.T.astype(f16)),             # [128,2,512]
        'wg2': _pack_kxm(Wg2.T.astype(f16)),           # [128,4,256]
        'wf1': _pack_kxm(Wf1.T.astype(f16)),           # [128,2,1024]
        'wf2': _pack_kxm(Wf2.T.astype(f16)),           # [128,8,256]
        'wb1': _pack_kxm(Wb1.T.astype(f16)),           # [128,2,64]
        'wb2': wb2_t,                                   # [64,9,64]
        'wb3': Wb3.T.astype(f16),                       # [64,256]
        'bt1': _pack_bias(t1),                          # [128,2] f32
        'bt2': _pack_bias(t2),
        'bbp': _pack_bias(bp),                          # [128,4]
        'bbf1': _pack_bias(bf1),                        # [128,8]
        'bbf2': _pack_bias(bf2),
        'btb1': np.ascontiguousarray(tb1[:, None].astype(np.float32)),  # [64,1]
        'btb2': np.ascontiguousarray(tb2[:, None].astype(np.float32)),
        'btb3': _pack_bias(tb3),
        'bsf': _pack_bias(sf),
        'btf': _pack_bias(tf),
        'bb3f': _pack_bias(sf * tb3),
    }


# --------------------------------------------------------------------------
# device kernel builder
# --------------------------------------------------------------------------
def _build_bass():
    import concourse.bass as bass
    import concourse.mybir as mybir
    from concourse import bacc
    from concourse.tile import TileContext
    from concourse.masks import make_identity

    dt = mybir.dt
    F16 = dt.float16
    F32 = dt.float32
    AF = mybir.ActivationFunctionType
    OP = mybir.AluOpType

    nc = bacc.Bacc()

    # ---- DRAM parameters ----
    x_d = nc.declare_dram_parameter("x", [IMGS_PER_CORE, C, N], F32, isOutput=False)
    wg1_d = nc.declare_dram_parameter("wg1", [128, 2, 256], F16, isOutput=False)
    wp_d = nc.declare_dram_parameter("wp", [128, 2, 512], F16, isOutput=False)
    wq_d = nc.declare_dram_parameter("wq", [128, 2, 512], F16, isOutput=False)
    wg2_d = nc.declare_dram_parameter("wg2", [128, 4, 256], F16, isOutput=False)
    wf1_d = nc.declare_dram_parameter("wf1", [128, 2, 1024], F16, isOutput=False)
    wf2_d = nc.declare_dram_parameter("wf2", [128, 8, 256], F16, isOutput=False)
    wb1_d = nc.declare_dram_parameter("wb1", [128, 2, 64], F16, isOutput=False)
    wb2_d = nc.declare_dram_parameter("wb2", [64, 9, 64], F16, isOutput=False)
    wb3_d = nc.declare_dram_parameter("wb3", [64, 256], F16, isOutput=False)
    bt1_d = nc.declare_dram_parameter("bt1", [128, 2], F32, isOutput=False)
    bt2_d = nc.declare_dram_parameter("bt2", [128, 2], F32, isOutput=False)
    bbp_d = nc.declare_dram_parameter("bbp", [128, 4], F32, isOutput=False)
    bbf1_d = nc.declare_dram_parameter("bbf1", [128, 8], F32, isOutput=False)
    bbf2_d = nc.declare_dram_parameter("bbf2", [128, 2], F32, isOutput=False)
    btb1_d = nc.declare_dram_parameter("btb1", [64, 1], F32, isOutput=False)
    btb2_d = nc.declare_dram_parameter("btb2", [64, 1], F32, isOutput=False)
    btb3_d = nc.declare_dram_parameter("btb3", [128, 2], F32, isOutput=False)
    bsf_d = nc.declare_dram_parameter("bsf", [128, 2], F32, isOutput=False)
    btf_d = nc.declare_dram_parameter("btf", [128, 2], F32, isOutput=False)
    bb3f_d = nc.declare_dram_parameter("bb3f", [128, 2], F32, isOutput=False)
    q_drams = [nc.dram_tensor(f"q_dram{i}", [N, 512], F16)
               for i in range(IMGS_PER_CORE)]
    out_d = nc.declare_dram_parameter("out", [IMGS_PER_CORE, C, N], F32, isOutput=True)

    with TileContext(nc) as tc:
        import contextlib
        ctx = contextlib.ExitStack()
        with ctx:
            consts = ctx.enter_context(tc.tile_pool(name="consts", bufs=1))
            pool_xc = ctx.enter_context(tc.tile_pool(name="xc", bufs=2))
            pool_feat = ctx.enter_context(tc.tile_pool(name="feat", bufs=2))
            pool_sm = ctx.enter_context(tc.tile_pool(name="sm", bufs=2))
            pool_simI = ctx.enter_context(tc.tile_pool(name="simI", bufs=3))
            pool_f1 = ctx.enter_context(tc.tile_pool(name="f1o", bufs=2))
            pool_idx = ctx.enter_context(tc.tile_pool(name="idx", bufs=2))
            pool_q = ctx.enter_context(tc.tile_pool(name="q", bufs=2))
            pool_p = ctx.enter_context(tc.tile_pool(name="p", bufs=2))
            pool_gath = ctx.enter_context(tc.tile_pool(name="gath", bufs=3))
            pool_e = ctx.enter_context(tc.tile_pool(name="e", bufs=2))
            pool_h = ctx.enter_context(tc.tile_pool(name="h", bufs=2))
            pool_bn = ctx.enter_context(tc.tile_pool(name="bn", bufs=2))
            pool_o = ctx.enter_context(tc.tile_pool(name="o", bufs=2))
            pool_tmp = ctx.enter_context(tc.tile_pool(name="tmp", bufs=2))
            psum = ctx.enter_context(tc.tile_pool(name="psum", bufs=3, space="PSUM"))
            psumT = ctx.enter_context(tc.tile_pool(name="psumT", bufs=2, space="PSUM"))
            psum1 = ctx.enter_context(tc.tile_pool(name="psum1", bufs=1, space="PSUM"))
            psum64 = ctx.enter_context(tc.tile_pool(name="psum64", bufs=2, space="PSUM"))

            def load(name, shape, dtype, src):
                t = consts.tile(shape, dtype, name=name)
                nc.sync.dma_start(out=t[:], in_=src[:])
                return t

            # wave 1: only what front(0) needs (keeps the sync queue clear
            # for q stores + wrap DMAs that gate the first gathers)
            wg1 = load("wg1s", [128, 2, 256], F16, wg1_d)
            bt1 = load("bt1s", [128, 2], F32, bt1_d)
            wq = load("wqs", [128, 2, 512], F16, wq_d)
            wp = load("wps", [128, 2, 512], F16, wp_d)
            wv = {}

            def load_wave2():
                wv['wg2'] = load("wg2s", [128, 4, 256], F16, wg2_d)
                wv['wf1'] = load("wf1s", [128, 2, 1024], F16, wf1_d)
                wv['wf2'] = load("wf2s", [128, 8, 256], F16, wf2_d)
                wv['wb1'] = load("wb1s", [128, 2, 64], F16, wb1_d)
                wv['wb2'] = load("wb2s", [64, 9, 64], F16, wb2_d)
                wv['wb3'] = load("wb3s", [64, 256], F16, wb3_d)
                wv['bt2'] = load("bt2s", [128, 2], F32, bt2_d)
                wv['bbp'] = load("bbps", [128, 4], F32, bbp_d)
                wv['bbf1'] = load("bbf1s", [128, 8], F32, bbf1_d)
                wv['bbf2'] = load("bbf2s", [128, 2], F32, bbf2_d)
                wv['btb1'] = load("btb1s", [64, 1], F32, btb1_d)
                wv['btb2'] = load("btb2s", [64, 1], F32, btb2_d)
                wv['btb3'] = load("btb3s", [128, 2], F32, btb3_d)
                wv['bsf'] = load("bsfs", [128, 2], F32, bsf_d)
                wv['btf'] = load("btfs", [128, 2], F32, btf_d)
                wv['bb3f'] = load("bb3fs", [128, 2], F32, bb3f_d)

            # SWDGE warm-up: a dummy gather pays the one-time pool-config
            # + ucode setup cost before the real gathers need the engine.
            warm_idx = consts.tile([128, 8], dt.int16, name="warm_idx")
            nc.vector.memset(warm_idx[:], 0)
            warm_out = consts.tile([128, 1, 512], F16, name="warm_out")
            nc.gpsimd.dma_gather(
                out_ap=warm_out[:], in_ap=q_drams[0][:], idxs_ap=warm_idx[:],
                num_idxs=128, num_idxs_reg=128, elem_size=512,
                transpose=False, single_packet=False)

            ident = consts.tile([128, 128], F16, name="ident")
            make_identity(nc, ident[:])
            negid = consts.tile([128, 128], F16, name="negid")
            nc.scalar.activation(out=negid[:], in_=ident[:], func=AF.Copy,
                                 scale=NEG_BIG)
            ones = consts.tile([128, 128], F16, name="ones")
            nc.vector.memset(ones[:], 1.0)
            # idbig[k, f] = 1 iff f == k + 384 (shifted identity for diag-kill)
            idbig = consts.tile([128, 1024], F16, name="idbig")
            nc.vector.memset(idbig[:], 0.0)
            nc.gpsimd.affine_select(
                out=idbig[:], in_=idbig[:],
                compare_op=mybir.AluOpType.not_equal, fill=1.0,
                base=384, pattern=[[-1, 1024]], channel_multiplier=1)

            # per-image state
            st = [dict() for _ in range(IMGS_PER_CORE)]

            # ============ phase functions ============
            def loadx(img):
                d = st[img]
                xc = pool_xc.tile([128, 2, N], F16, name=f"xc{img}", tag="xc")
                if img == 0:
                    for t in range(2):
                        # cast f32->f16 during DMA (SWDGE queue idle at t=0)
                        nc.gpsimd.dma_start(out=xc[:, t, :],
                                            in_=x_d[img, t * 128:(t + 1) * 128, :])
                else:
                    # keep the SWDGE queue free for img0's gathers
                    for t in range(2):
                        x32 = pool_xc.tile([128, N], F32, name=f"x32_{t}",
                                           tag="x32")
                        nc.sync.dma_start(out=x32[:],
                                          in_=x_d[img, t * 128:(t + 1) * 128, :])
                        if t == 0:
                            nc.scalar.activation(out=xc[:, t, :], in_=x32[:],
                                                 func=AF.Copy)
                        else:
                            nc.vector.tensor_copy(out=xc[:, t, :], in_=x32[:])
                wrapped = pool_idx.tile([128, 8, 64], dt.int16,
                                        name=f"wrapped{img}", tag="wrapped")
                d['xc'] = xc
                d['wrapped'] = wrapped

            def front(img):
                d = st[img]
                xc = d['xc']
                # ---- g1 -> featT (unnormalized) + fsq (Square evac) ----
                featT = pool_feat.tile([128, 2, N], F16, name=f"featT{img}", tag="featT")
                fsq = pool_sm.tile([128, 2, N], F16, name=f"fsq{img}", tag="fsq")
                for to in range(2):
                    for nb in range(2):
                        ps = psum.tile([128, 512], F32, name="ps_g1", tag="ps")
                        for kt in range(2):
                            nc.tensor.matmul(
                                ps[:], lhsT=wg1[:, kt, to * 128:(to + 1) * 128],
                                rhs=xc[:, kt, nb * 512:(nb + 1) * 512],
                                start=(kt == 0), stop=(kt == 1))
                        sl = slice(nb * 512, (nb + 1) * 512)
                        nc.scalar.activation(
                            out=fsq[:, to, sl], in_=ps[:],
                            func=AF.Square, bias=bt1[:, to:to + 1])
                        nc.scalar.activation(
                            out=featT[:, to, sl], in_=ps[:],
                            func=AF.Identity, bias=bt1[:, to:to + 1])

                # ---- n2 partial (matmul + fast reciprocal) ----
                invn = pool_sm.tile([1, N], F16, name=f"invn{img}", tag="invn")
                rn2 = pool_sm.tile([1, N], F32, name=f"rn2{img}", tag="rn2")
                for nb in range(2):
                    ps1 = psum1.tile([1, 512], F32, name="ps_n2", tag="ps1")
                    for kt in range(2):
                        nc.tensor.matmul(
                            ps1[:], lhsT=ones[:, 0:1],
                            rhs=fsq[:, kt, nb * 512:(nb + 1) * 512],
                            start=(kt == 0), stop=(kt == 1))
                    nc.vector.reciprocal_approx_fast(
                        out=rn2[:, nb * 512:(nb + 1) * 512], in_=ps1[:])

                # ---- invn/invnb broadcast (feeds sim evac; keep early) ----
                nc.scalar.activation(out=invn[:], in_=rn2[:], func=AF.Sqrt)
                invnb = pool_sm.tile([128, N], F16, name=f"invnb{img}", tag="invnb")
                for nb in range(2):
                    psb = psum.tile([128, 512], F32, name="ps_bc", tag="ps")
                    nc.tensor.matmul(psb[:], lhsT=ones[0:1, :],
                                     rhs=invn[:, nb * 512:(nb + 1) * 512],
                                     start=True, stop=True)
                    nc.scalar.activation(out=invnb[:, nb * 512:(nb + 1) * 512],
                                         in_=psb[:], func=AF.Copy)

                # ---- p (node-partitioned, no bias yet) ----
                p_np = pool_p.tile([128, 8, 512], F16, name=f"p_np{img}",
                                   tag="p_np")
                for nt in range(8):
                    ps = psum.tile([128, 512], F32, name="ps_p", tag="ps")
                    for kt in range(2):
                        nc.tensor.matmul(
                            ps[:], lhsT=featT[:, kt, nt * 128:(nt + 1) * 128],
                            rhs=wp[:, kt, :], start=(kt == 0), stop=(kt == 1))
                    nc.scalar.activation(out=p_np[:, nt, :], in_=ps[:],
                                         func=AF.Copy)
                d['p_np'] = p_np

                # ---- q (node-partitioned) -> q_sb + q_dram (gates gathers) ----
                q_dram = q_drams[img]
                q_sb = pool_q.tile([128, 8, 512], F16, name=f"q_sb{img}",
                                   tag="q_sb")
                for nt in range(8):
                    ps = psum.tile([128, 512], F32, name="ps_q", tag="ps")
                    for kt in range(2):
                        nc.tensor.matmul(
                            ps[:], lhsT=featT[:, kt, nt * 128:(nt + 1) * 128],
                            rhs=wq[:, kt, :], start=(kt == 0), stop=(kt == 1))
                    nc.scalar.activation(out=q_sb[:, nt, :], in_=ps[:],
                                         func=AF.Copy)
                    nc.sync.dma_start(out=q_dram[nt * 128:(nt + 1) * 128, :],
                                      in_=q_sb[:, nt, :])
                d['q_sb'] = q_sb

                # ---- sim scores (diag killed), scaled by invnb on evac ----
                ixbuf = pool_idx.tile([128, 8, 8], dt.uint16,
                                      name=f"ixbuf{img}", tag="ixbuf")
                ixi = ixbuf[:].bitcast(dt.int16)
                wrapped = d['wrapped']
                # wrapped[p16, I, 8k+g] = ixbuf[16g+p16, I, k]
                wview = wrapped[0:16, :, :].rearrange(
                    "p i (k g) -> p i k g", k=8, g=8)
                wflat = wrapped[:].rearrange("p a b -> p (a b)")
                for I in range(8):
                    simI = pool_simI.tile([128, N], F16, name="simI", tag="simI")
                    for cb in range(2):
                        has_diag = (cb == I // 4)
                        ps = psum.tile([128, 512], F32, name="ps_sim", tag="ps")
                        for kt in range(2):
                            nc.tensor.matmul(
                                ps[:], lhsT=featT[:, kt, I * 128:(I + 1) * 128],
                                rhs=featT[:, kt, cb * 512:(cb + 1) * 512],
                                start=(kt == 0),
                                stop=(kt == 1 and not has_diag))
                        if has_diag:
                            off = I * 128 - cb * 512
                            nc.tensor.matmul(ps[:], lhsT=negid[:],
                                             rhs=idbig[:, 384 - off:896 - off],
                                             start=False, stop=True)
                        # evac fused with per-column 1/||f_m|| scale
                        nc.vector.tensor_mul(
                            simI[:, cb * 512:(cb + 1) * 512], ps[:],
                            invnb[:, cb * 512:(cb + 1) * 512])
                    mx = pool_tmp.tile([128, 8], F16, name="mx", tag="mx")
                    nc.vector.max(out=mx[:], in_=simI[:])
                    nc.vector.max_index(out=ixbuf[:, I, :],
                                        in_max=mx[:],
                                        in_values=simI[:])
                    if I % 2 == 1:
                        # wrap this I-pair into the 16-partition layout
                        isl = slice(I - 1, I + 1)
                        for g in range(8):
                            nc.sync.dma_start(
                                out=wview[:, isl, :, g],
                                in_=ixi[16 * g:16 * (g + 1), isl, :])
                        # log-doubling broadcast to 128 partitions
                        csl = slice(64 * (I - 1), 64 * (I + 1))
                        nc.sync.dma_start(out=wflat[16:32, csl],
                                          in_=wflat[0:16, csl])
                        nc.sync.dma_start(out=wflat[32:64, csl],
                                          in_=wflat[0:32, csl])
                        nc.sync.dma_start(out=wflat[64:128, csl],
                                          in_=wflat[0:64, csl])
                        gather_issue(img, I - 1)
                        gather_issue(img, I)




            def gather_issue(img, I):
                """Issue the NT SWDGE gather for 128-node block I."""
                d = st[img]
                wrapped = d['wrapped']
                wflat = wrapped[:].rearrange("p a b -> p (a b)")
                go = pool_gath.tile([128, 8, 512], F16, name="go", tag="go")
                nc.gpsimd.dma_gather(
                    out_ap=go[:], in_ap=q_drams[img][:],
                    idxs_ap=wflat[:, 64 * I:64 * (I + 1)],
                    num_idxs=1024, num_idxs_reg=1024, elem_size=512,
                    transpose=False, single_packet=False)
                d[f'go{I}'] = go

            def fold_join(img, I):
                """Max fold over the 8 gathered rows + self + p join."""
                d = st[img]
                p_np = d['p_np']
                q_sb = d['q_sb']
                go = d.pop(f'go{I}')
                gf = go[:].rearrange("p a b -> p (a b)")
                nc.vector.tensor_max(gf[:, 2048:4096], gf[:, 0:2048],
                                     gf[:, 2048:4096])
                nc.vector.tensor_max(gf[:, 3072:4096], gf[:, 2048:3072],
                                     gf[:, 3072:4096])
                nc.vector.tensor_max(gf[:, 3584:4096], gf[:, 3072:3584],
                                     gf[:, 3584:4096])
                # self neighbor + p join; bias+relu happen in transpose evac
                nc.vector.tensor_max(gf[:, 3584:4096], gf[:, 3584:4096],
                                     q_sb[:, I, :])
                nc.vector.tensor_add(p_np[:, I, :], p_np[:, I, :],
                                     gf[:, 3584:4096])

            def transpose_e(img, h):
                """PE-transpose e half h (nodes 512h..512h+512) into eT
                [128c, 4cb, N]; evac applies edge bias + relu per channel."""
                d = st[img]
                p_np = d['p_np']
                if h == 0:
                    d['eT'] = pool_e.tile([128, 4, N], F16, name=f"eT{img}",
                                          tag="eT")
                    # xc' = xc + bt2 : lets the g2 join skip its scalar evac
                    xcp = pool_xc.tile([128, 2, N], F16, name=f"xcp{img}",
                                       tag="xcp")
                    for to in range(2):
                        nc.scalar.activation(out=xcp[:, to, :],
                                             in_=d['xc'][:, to, :],
                                             func=AF.Identity,
                                             bias=wv['bt2'][:, to:to + 1])
                    d['xcp'] = xcp
                eT = d['eT']
                for cb in range(4):
                    psT = psumT.tile([128, 512], F16, name="psT", tag="psT")
                    for j in range(4):
                        I = 4 * h + j
                        nc.tensor.transpose(
                            out=psT[:, j * 128:(j + 1) * 128],
                            in_=p_np[:, I, cb * 128:(cb + 1) * 128],
                            identity=ident[:])
                    nc.scalar.activation(
                        out=eT[:, cb, h * 512:(h + 1) * 512], in_=psT[:],
                        func=AF.Relu, bias=wv['bbp'][:, cb:cb + 1])

            def tail_gf(img, nb):
                """g2 + residual, f1, f2 + residual for one node-half."""
                d = st[img]
                xc = d['xc']
                eT = d['eT']
                sl = slice(nb * 512, (nb + 1) * 512)
                if nb == 0:
                    d['h'] = pool_h.tile([128, 2, N], F16, name=f"h{img}", tag="h")
                h = d['h']
                f1o = pool_f1.tile([128, 8, 512], F16, name="f1o", tag="f1o")
                # g2 + residual (bias pre-folded into xcp)
                xcp = d['xcp']
                for to in range(2):
                    ps = psum.tile([128, 512], F32, name="ps_g2", tag="ps")
                    for kt in range(4):
                        nc.tensor.matmul(
                            ps[:], lhsT=wv['wg2'][:, kt, to * 128:(to + 1) * 128],
                            rhs=eT[:, kt, sl],
                            start=(kt == 0), stop=(kt == 3))
                    tmp = pool_tmp.tile([128, 512], F16, name="g2tmp", tag="evtmp")
                    nc.scalar.activation(out=tmp[:], in_=ps[:], func=AF.Copy)
                    nc.vector.tensor_add(h[:, to, sl], tmp[:], xcp[:, to, sl])
                # f1 (relu evac)
                for to in range(8):
                    ps = psum.tile([128, 512], F32, name="ps_f1", tag="ps")
                    for kt in range(2):
                        nc.tensor.matmul(
                            ps[:], lhsT=wv['wf1'][:, kt, to * 128:(to + 1) * 128],
                            rhs=h[:, kt, sl],
                            start=(kt == 0), stop=(kt == 1))
                    nc.scalar.activation(
                        out=f1o[:, to, :], in_=ps[:],
                        func=AF.Relu, bias=wv['bbf1'][:, to:to + 1])
                # f2 + residual (h -> h2 in place)
                for to in range(2):
                    ps = psum.tile([128, 512], F32, name="ps_f2", tag="ps")
                    for kt in range(8):
                        nc.tensor.matmul(
                            ps[:], lhsT=wv['wf2'][:, kt, to * 128:(to + 1) * 128],
                            rhs=f1o[:, kt, :],
                            start=(kt == 0), stop=(kt == 7))
                    tmp = pool_tmp.tile([128, 512], F16, name="f2tmp", tag="evtmp")
                    nc.scalar.activation(out=tmp[:], in_=ps[:],
                                         func=AF.Identity, bias=wv['bbf2'][:, to:to + 1])
                    nc.vector.tensor_add(h[:, to, sl], tmp[:], h[:, to, sl])

            def tail_bneck(img):
                """bottleneck + final BN + store (whole image)."""
                d = st[img]
                xc = d['xc']
                h = d['h']  # h2 now
                b1o = pool_bn.tile([64, N], F16, name=f"b1o{img}", tag="b1o")
                for nb in range(2):
                    ps = psum64.tile([64, 512], F32, name="ps_b1", tag="ps64")
                    for kt in range(2):
                        nc.tensor.matmul(
                            ps[:], lhsT=wv['wb1'][:, kt, :],
                            rhs=h[:, kt, nb * 512:(nb + 1) * 512],
                            start=(kt == 0), stop=(kt == 1))
                    nc.scalar.activation(out=b1o[:, nb * 512:(nb + 1) * 512],
                                         in_=ps[:], func=AF.Relu, bias=wv['btb1'][:, 0:1])
                pad = pool_bn.tile([64, 34 * 34], F16, name=f"pad{img}", tag="pad")
                nc.vector.memset(pad[:], 0.0)
                pad3 = pad[:].rearrange("p (r c) -> p r c", r=34)
                b1v = b1o[:].rearrange("p (r c) -> p r c", r=32)
                nc.vector.tensor_copy(pad3[:, 1:33, 1:33], b1v)
                # D = bsf*(h2 + btb3) + (bsf*xc + btf): precomputed during
                # b1/b2 so the b3 join is evac-scale + one add
                Dt = pool_xc.tile([128, 2, N], F16, name=f"D{img}", tag="xcp")
                for to in range(2):
                    nc.scalar.activation(out=Dt[:, to, :], in_=h[:, to, :],
                                         func=AF.Identity,
                                         scale=wv['bsf'][:, to:to + 1],
                                         bias=wv['bb3f'][:, to:to + 1])
                    Bt = pool_tmp.tile([128, N], F16, name="Bt", tag="evtmp")
                    nc.scalar.activation(out=Bt[:], in_=xc[:, to, :],
                                         func=AF.Identity,
                                         scale=wv['bsf'][:, to:to + 1],
                                         bias=wv['btf'][:, to:to + 1])
                    nc.vector.tensor_add(Dt[:, to, :], Dt[:, to, :], Bt[:])

                b2o = pool_bn.tile([64, N], F16, name=f"b2o{img}", tag="b2o")
                for nb in range(2):
                    ps = psum64.tile([64, 512], F32, name="ps_b2", tag="ps64")
                    for tap in range(9):
                        dy, dx = tap // 3, tap % 3
                        rhs = pad3[:, 16 * nb + dy:16 * nb + dy + 16, dx:dx + 32]
                        nc.tensor.matmul(ps[:], lhsT=wv['wb2'][:, tap, :], rhs=rhs,
                                         start=(tap == 0), stop=(tap == 8))
                    nc.scalar.activation(out=b2o[:, nb * 512:(nb + 1) * 512],
                                         in_=ps[:], func=AF.Relu, bias=wv['btb2'][:, 0:1])
                for to in range(2):
                    out32 = pool_o.tile([128, N], F32, name="out32", tag="o32")
                    d[f'out32_{to}'] = out32
                    for nb in range(2):
                        ps = psum.tile([128, 512], F32, name="ps_b3", tag="ps")
                        nc.tensor.matmul(
                            ps[:], lhsT=wv['wb3'][:, to * 128:(to + 1) * 128],
                            rhs=b2o[:, nb * 512:(nb + 1) * 512],
                            start=True, stop=True)
                        sl = slice(nb * 512, (nb + 1) * 512)
                        tmp = pool_tmp.tile([128, 512], F32, name="b3tmp", tag="evtmp")
                        nc.scalar.activation(out=tmp[:], in_=ps[:],
                                             func=AF.Identity,
                                             scale=wv['bsf'][:, to:to + 1])
                        nc.vector.tensor_add(out32[:, sl], tmp[:],
                                             Dt[:, to, sl])
                    nc.sync.dma_start(out=out_d[img, to * 128:(to + 1) * 128, :],
                                      in_=out32[:])

            # ============ emission (software pipeline over 2 images) ============
            loadx(0)
            front(0)
            load_wave2()
            loadx(1)
            front(1)
            for I in range(8):
                fold_join(0, I)
            transpose_e(0, 0)
            tail_gf(0, 0)
            transpose_e(0, 1)
            tail_gf(0, 1)
            for I in range(4):
                fold_join(1, I)
            tail_bneck(0)
            for I in range(4, 8):
                fold_join(1, I)
            transpose_e(1, 0)
            tail_gf(1, 0)
            transpose_e(1, 1)
            tail_gf(1, 1)
            tail_bneck(1)

    nc.finalize()
    return nc


# --------------------------------------------------------------------------
# entry point
# --------------------------------------------------------------------------
def kernel(**inputs):
    inp = {k: np.asarray(v) for k, v in inputs.items()}
    w = _prep_weights(inp)

    if 'nc' not in _cache:
        _cache['nc'] = _build_bass()
    nc = _cache['nc']

    x = inp['x'].astype(np.float32).reshape(B, C, N)
    in_maps = []
    for c in range(N_CORES):
        m = {'x': np.ascontiguousarray(x[c * 2:(c + 1) * 2])}
        m.update({k: v for k, v in w.items()})
        in_maps.append(m)

    from concourse.bass_utils import run_bass_kernel_spmd
    trace = bool(os.environ.get("KBENCH_TRACE"))
    res = run_bass_kernel_spmd(nc, in_maps, core_ids=list(range(N_CORES)),
                               trace=trace)
    _cache['exec_time_ns'] = res.exec_time_ns
    _cache['results'] = res
    out = np.zeros((B, C, N), np.float32)
    for c in range(N_CORES):
        out[c * 2:(c + 1) * 2] = res.results[c]['out']
    return out.reshape(B, C, H, W)
